# revision 12
# baseline (speedup 1.0000x reference)
"""Fused LayerNorm + single-head attention (q/k dim 128, v dim 512) for TRN2.

Problem: nn_MHAttention (B=16, S=2048, D=512): for each batch b,
  xn = LayerNorm(x[b]); q = xn@Wq.T; k = xn@Wk.T; v = xn@Wv.T
  out[b] = softmax(q@k.T / sqrt(512)) @ v

Strategy: data-parallel over batch across 8 NeuronCores (2 batches/core).
Per batch, everything stays on-chip (flash-style, no S^2 HBM traffic):
  - load x tiles [128s, 512d], LayerNorm in natural orientation (bn_stats),
  - PE-transpose normalized tiles -> tT [d, s],
  - qT/kT = W @ tT (weights pre-transposed host-side, softmax scale and
    ln_w folded into Wq host-side),  v in natural [s, o] layout,
  - scoresT[sk, sq] = kT_chunk.T @ qT (transposed scores so that the
    attn@v contraction needs no on-chip transposes),
  - exp on ScalarE (scores are O(1), max-subtraction provably unneeded),
  - out[sq, o] += pT_chunk.T @ v accumulated over sk in PSUM, with a
    ones-column appended to v so the same matmuls accumulate the softmax
    row-sums for free; final 1/rowsum scale on VectorE.
Matmuls use float32r (full PE rate for moving dim >= 256, ~1e-4 matmul
rel-err vs 2e-3 for bf16).
"""

import math

import numpy as np

B, S, D, H, O = 16, 2048, 512, 128, 512
N_CORES = 8
B_PER_CORE = B // N_CORES
P = 128          # partitions
CHUNK = 256      # sq chunk width for the attention phase
STAGE = 4        # s-tiles per x-load DMA group

_cache = {}


def _build_nc(with_bias: bool, b_per_core: int, s_tiles: int):
    import concourse.bass as bass
    import concourse.mybir as mybir
    import concourse.tile as tile
    from concourse import bacc
    from concourse.masks import make_identity

    f32 = mybir.dt.float32
    f32r = mybir.dt.float32r
    AF = mybir.ActivationFunctionType
    Alu = mybir.AluOpType
    Ax = mybir.AxisListType

    s_len = s_tiles * P
    d_tiles = D // P
    n_chunks = s_len // CHUNK
    subs = CHUNK // P
    assert s_tiles % STAGE == 0 and s_len % CHUNK == 0

    nc = bacc.Bacc("TRN2", target_bir_lowering=False, debug=False,
                   num_devices=N_CORES)
    x_d = nc.dram_tensor("x", [b_per_core, s_len, D], f32, kind="ExternalInput")
    wqt_d = nc.dram_tensor("wqt", [P, d_tiles, H], f32, kind="ExternalInput")
    wkt_d = nc.dram_tensor("wkt", [P, d_tiles, H], f32, kind="ExternalInput")
    wvt_d = nc.dram_tensor("wvt", [P, d_tiles, O], f32, kind="ExternalInput")
    if with_bias:
        bq_d = nc.dram_tensor("bq", [H, 1], f32, kind="ExternalInput")
        bk_d = nc.dram_tensor("bk", [H, 1], f32, kind="ExternalInput")
        bv_d = nc.dram_tensor("bv", [1, O], f32, kind="ExternalInput")
    out_d = nc.dram_tensor("out", [b_per_core, s_len, O], f32,
                           kind="ExternalOutput")

    with tile.TileContext(nc) as tc:
        from contextlib import ExitStack
        with ExitStack() as ctx:
            consts = ctx.enter_context(tc.tile_pool(name="consts", bufs=1))
            stagep = ctx.enter_context(tc.tile_pool(name="stage", bufs=2))
            ttp = ctx.enter_context(tc.tile_pool(name="tt", bufs=1))
            qkp = ctx.enter_context(tc.tile_pool(name="qk", bufs=1))
            vp = ctx.enter_context(tc.tile_pool(name="v", bufs=1))
            ptp = ctx.enter_context(tc.tile_pool(name="pt", bufs=2))
            outp = ctx.enter_context(tc.tile_pool(name="outsb", bufs=1))
            small = ctx.enter_context(tc.tile_pool(name="small", bufs=4))
            psA = ctx.enter_context(tc.tile_pool(name="psA", bufs=4, space="PSUM"))
            psO = ctx.enter_context(tc.tile_pool(name="psO", bufs=2, space="PSUM"))

            eps_t = consts.tile([P, 1], f32)
            nc.vector.memset(eps_t[:], 1e-5)
            ident = consts.tile([P, P], f32)
            make_identity(nc, ident[:])
            # f32r matmul operands must come from a rounding instruction, so
            # weights are DMA'd into staging and rounded into their tiles.
            wqt = consts.tile([P, d_tiles, H], f32)
            wkt = consts.tile([P, d_tiles, H], f32)
            wvt = consts.tile([P, d_tiles, O], f32)
            for w_t, w_d in ((wqt, wqt_d), (wkt, wkt_d), (wvt, wvt_d)):
                wst = stagep.tile([P, d_tiles, O], f32, tag="stage")
                fs = w_t.shape[2]
                nc.sync.dma_start(out=wst[:, :, 0:fs], in_=w_d[:])
                nc.vector.tensor_copy(out=w_t[:].bitcast(f32r),
                                      in_=wst[:, :, 0:fs])
            if with_bias:
                bq_t = consts.tile([H, 1], f32)
                nc.sync.dma_start(out=bq_t[:], in_=bq_d[:])
                bk_t = consts.tile([H, 1], f32)
                nc.sync.dma_start(out=bk_t[:], in_=bk_d[:])
                bv_t = consts.tile([P, O], f32)
                nc.sync.dma_start(out=bv_t[:], in_=bv_d[:].broadcast_to((P, O)))

            for b in range(b_per_core):
                xb = x_d[b].rearrange("(t p) d -> p t d", p=P)
                ob = out_d[b].rearrange("(t p) o -> p t o", p=P)

                tT = ttp.tile([P, d_tiles, s_len], f32, tag="tt")
                qT = qkp.tile([P, s_len], f32, tag="q")
                kT = qkp.tile([P, s_len], f32, tag="k")
                # v with an extra all-ones column: the attn@v matmul then
                # accumulates the softmax row-sums for free.
                # two ones-columns keep the second attn@v matmul's moving
                # dim even (odd free dims fail the f32r ISA check)
                v_sb = vp.tile([P, s_tiles, O + 2], f32, tag="v")
                vones = small.tile([P, s_tiles, 2], f32, tag="vones")
                nc.vector.memset(vones[:], 1.0)
                nc.vector.tensor_copy(out=v_sb[:, :, O:O + 2].bitcast(f32r),
                                      in_=vones[:])
                out_sb = outp.tile([P, s_tiles, O], f32, tag="o")

                # ---- Phase A: load, LayerNorm, transpose, projections ----
                for g in range(s_tiles // STAGE):
                    stg = stagep.tile([P, STAGE, D], f32, tag="stage")
                    nc.sync.dma_start(out=stg[:],
                                      in_=xb[:, g * STAGE:(g + 1) * STAGE, :])
                    for j in range(STAGE):
                        st = g * STAGE + j
                        sv = stg[:, j, :]
                        stats = small.tile([P, 6], f32, tag="stats")
                        nc.vector.bn_stats(out=stats[:], in_=sv)
                        mv = small.tile([P, 2], f32, tag="mv")
                        nc.vector.bn_aggr(out=mv[:], in_=stats[:])
                        rstd = small.tile([P, 1], f32, tag="rstd")
                        nc.scalar.activation(out=rstd[:], in_=mv[:, 1:2],
                                             func=AF.Sqrt, bias=eps_t[:])
                        nc.vector.reciprocal(out=rstd[:], in_=rstd[:])
                        nc.vector.tensor_scalar(
                            out=sv, in0=sv, scalar1=mv[:, 0:1], scalar2=rstd[:],
                            op0=Alu.subtract, op1=Alu.mult)
                        for dt in range(d_tiles):
                            ps = psA.tile([P, P], f32, tag="a")
                            nc.tensor.transpose(
                                ps[:], sv[:, dt * P:(dt + 1) * P], ident[:])
                            nc.vector.tensor_copy(
                                out=tT[:, dt, st * P:(st + 1) * P].bitcast(f32r),
                                in_=ps[:])

                # qT / kT projections (chunks of 512 along s)
                for c in range(s_len // 512):
                    cs = slice(c * 512, (c + 1) * 512)
                    for name, w_t, dst in (("q", wqt, qT), ("k", wkt, kT)):
                        ps = psA.tile([P, 512], f32, tag="a")
                        for dt in range(d_tiles):
                            nc.tensor.matmul(
                                ps[:], w_t[:, dt, :].bitcast(f32r),
                                tT[:, dt, cs].bitcast(f32r),
                                start=(dt == 0), stop=(dt == d_tiles - 1))
                        if with_bias:
                            bias = bq_t if name == "q" else bk_t
                            nc.vector.tensor_scalar_add(
                                out=dst[:, cs].bitcast(f32r), in0=ps[:],
                                scalar1=bias[:])
                        else:
                            nc.vector.tensor_copy(out=dst[:, cs].bitcast(f32r),
                                                  in_=ps[:])

                # v projection
                for st in range(s_tiles):
                    ps = psA.tile([P, O], f32, tag="a")
                    for dt in range(d_tiles):
                        nc.tensor.matmul(
                            ps[:], tT[:, dt, st * P:(st + 1) * P].bitcast(f32r),
                            wvt[:, dt, :].bitcast(f32r),
                            start=(dt == 0), stop=(dt == d_tiles - 1))
                    nc.vector.tensor_copy(out=v_sb[:, st, 0:O].bitcast(f32r), in_=ps[:])

                # ---- Phase B: attention over sq chunks ----
                HALF = O // 2
                for c in range(n_chunks):
                    cq = slice(c * CHUNK, (c + 1) * CHUNK)
                    pT = ptp.tile([P, s_tiles, CHUNK], f32, tag="pt")
                    for i in range(s_tiles):
                        sps = psA.tile([P, CHUNK], f32, tag="a")
                        nc.tensor.matmul(
                            sps[:], kT[:, i * P:(i + 1) * P].bitcast(f32r),
                            qT[:, cq].bitcast(f32r), start=True, stop=True)
                        nc.scalar.activation(out=pT[:, i, :].bitcast(f32r),
                                             in_=sps[:], func=AF.Exp)
                    for sub in range(subs):
                        st = c * subs + sub
                        # o split 256 + 257: the 257th column of the second
                        # half is p @ ones = the softmax row-sum.
                        ops_a = psO.tile([P, HALF], f32, tag="oa")
                        ops_b = psO.tile([P, HALF + 2], f32, tag="ob")
                        for i in range(s_tiles):
                            lhsT = pT[:, i, sub * P:(sub + 1) * P].bitcast(f32r)
                            nc.tensor.matmul(
                                ops_a[:], lhsT,
                                v_sb[:, i, 0:HALF].bitcast(f32r),
                                start=(i == 0), stop=(i == s_tiles - 1))
                            nc.tensor.matmul(
                                ops_b[:], lhsT,
                                v_sb[:, i, HALF:O + 2].bitcast(f32r),
                                start=(i == 0), stop=(i == s_tiles - 1))
                        recip = small.tile([P, 1], f32, tag="recip")
                        nc.vector.reciprocal(out=recip[:],
                                             in_=ops_b[:, HALF:HALF + 1])
                        nc.vector.tensor_scalar_mul(
                            out=out_sb[:, st, 0:HALF], in0=ops_a[:],
                            scalar1=recip[:])
                        nc.vector.tensor_scalar_mul(
                            out=out_sb[:, st, HALF:O], in0=ops_b[:, 0:HALF],
                            scalar1=recip[:])
                        if with_bias:
                            nc.vector.tensor_add(out=out_sb[:, st, :],
                                                 in0=out_sb[:, st, :],
                                                 in1=bv_t[:])
                    # store per half-batch
                    if (c + 1) % (n_chunks // 2) == 0:
                        h0 = (s_tiles // 2) * ((c + 1) // (n_chunks // 2) - 1)
                        hs = slice(h0, h0 + s_tiles // 2)
                        nc.sync.dma_start(out=ob[:, hs, :],
                                          in_=out_sb[:, hs, :])
    nc.compile()
    return nc


def _get_runner(with_bias: bool, b_per_core: int = B_PER_CORE,
                s_tiles: int = S // P):
    key = (with_bias, b_per_core, s_tiles)
    if key in _cache:
        return _cache[key]

    import jax
    import jax.numpy as jnp  # noqa: F401
    import concourse.mybir as mybir
    from concourse.bass2jax import (
        _bass_exec_p, install_neuronx_cc_hook, partition_id_tensor)
    from jax.experimental.shard_map import shard_map
    from jax.sharding import Mesh, PartitionSpec

    nc = _build_nc(with_bias, b_per_core, s_tiles)
    install_neuronx_cc_hook()
    partition_name = (nc.partition_id_tensor.name
                      if nc.partition_id_tensor else None)

    in_names, out_names, out_avals = [], [], []
    for alloc in nc.m.functions[0].allocations:
        if not isinstance(alloc, mybir.MemoryLocationSet):
            continue
        name = alloc.memorylocations[0].name
        if alloc.kind == "ExternalInput":
            if name != partition_name:
                in_names.append(name)
        elif alloc.kind == "ExternalOutput":
            out_names.append(name)
            out_avals.append(jax.core.ShapedArray(
                tuple(alloc.tensor_shape), mybir.dt.np(alloc.dtype)))
    n_params = len(in_names)
    all_in_names = list(in_names) + list(out_names)
    if partition_name is not None:
        all_in_names.append(partition_name)
    donate = tuple(range(n_params, n_params + len(out_names)))

    def _body(*args):
        operands = list(args)
        if partition_name is not None:
            operands.append(partition_id_tensor())
        outs = _bass_exec_p.bind(
            *operands, out_avals=tuple(out_avals),
            in_names=tuple(all_in_names),
            out_names=tuple(out_names), lowering_input_output_aliases=(),
            sim_require_finite=True, sim_require_nnan=True, nc=nc)
        return tuple(outs)

    devices = jax.devices()[:N_CORES]
    mesh = Mesh(np.asarray(devices), ("core",))
    nin = n_params + len(out_names)
    sharded = jax.jit(
        shard_map(_body, mesh=mesh, in_specs=(PartitionSpec("core"),) * nin,
                  out_specs=(PartitionSpec("core"),) * len(out_names),
                  check_rep=False),
        donate_argnums=donate, keep_unused=True)

    runner = (sharded, in_names, out_names, out_avals)
    _cache[key] = runner
    return runner


def _prep_weights(ln_w, ln_b, Wq, Wk, Wv):
    scale = np.float32(1.0 / math.sqrt(O))
    wq_eff = (Wq * ln_w[None, :]).astype(np.float32) * scale
    wk_eff = (Wk * ln_w[None, :]).astype(np.float32)
    wv_eff = (Wv * ln_w[None, :]).astype(np.float32)
    bq = (Wq.astype(np.float64) @ ln_b.astype(np.float64)).astype(np.float32) * scale
    bk = (Wk.astype(np.float64) @ ln_b.astype(np.float64)).astype(np.float32)
    bv = (Wv.astype(np.float64) @ ln_b.astype(np.float64)).astype(np.float32)
    with_bias = bool(np.any(bq) or np.any(bk) or np.any(bv))

    d_tiles = D // P
    def pack(w_eff, n_out):  # [n_out, D] -> [P, d_tiles, n_out]
        return np.ascontiguousarray(
            w_eff.T.reshape(d_tiles, P, n_out).transpose(1, 0, 2))

    packed = {
        "wqt": pack(wq_eff, H),
        "wkt": pack(wk_eff, H),
        "wvt": pack(wv_eff, O),
    }
    if with_bias:
        packed["bq"] = bq.reshape(H, 1)
        packed["bk"] = bk.reshape(H, 1)
        packed["bv"] = bv.reshape(1, O)
    return packed, with_bias


def run_device(x_full, packed, with_bias, b_per_core=B_PER_CORE,
               s_tiles=S // P):
    """x_full: [n_cores*b_per_core, s, D] -> out of same shape."""
    sharded, in_names, out_names, out_avals = _get_runner(
        with_bias, b_per_core, s_tiles)
    per_input = {"x": np.ascontiguousarray(x_full, dtype=np.float32)}
    for name, arr in packed.items():
        per_input[name] = np.concatenate([arr] * N_CORES, axis=0)
    args = [per_input[n] for n in in_names]
    zero_outs = [np.zeros((N_CORES * a.shape[0], *a.shape[1:]), a.dtype)
                 for a in out_avals]
    out_arrs = sharded(*args, *zero_outs)
    out = np.asarray(out_arrs[out_names.index("out")])
    return out.reshape(x_full.shape[0], x_full.shape[1], O)


def kernel(x, ln_w, ln_b, Wq, Wk, Wv):
    x = np.asarray(x, dtype=np.float32)
    packed, with_bias = _prep_weights(
        np.asarray(ln_w, np.float32), np.asarray(ln_b, np.float32),
        np.asarray(Wq, np.float32), np.asarray(Wk, np.float32),
        np.asarray(Wv, np.float32))
    return run_device(x, packed, with_bias)


# revision 13
# speedup vs baseline: 1.6231x; 1.6231x over previous
"""Fused LayerNorm + single-head attention (q/k dim 128, v dim 512) for TRN2.

Problem: nn_MHAttention (B=16, S=2048, D=512): for each batch b,
  xn = LayerNorm(x[b]); q = xn@Wq.T; k = xn@Wk.T; v = xn@Wv.T
  out[b] = softmax(q@k.T / sqrt(512)) @ v

Strategy: data-parallel over batch across 8 NeuronCores (2 batches/core).
Per batch, everything stays on-chip (flash-style, no S^2 HBM traffic):
  - load x tiles [128s, 512d], LayerNorm in natural orientation (bn_stats),
  - PE-transpose normalized tiles -> tT [d, s],
  - qT/kT = W @ tT (weights pre-transposed host-side, softmax scale and
    ln_w folded into Wq host-side),  v in natural [s, o] layout,
  - scoresT[sk, sq] = kT_chunk.T @ qT (transposed scores so that the
    attn@v contraction needs no on-chip transposes),
  - exp on ScalarE (scores are O(1), max-subtraction provably unneeded),
  - out[sq, o] += pT_chunk.T @ v accumulated over sk in PSUM, with a
    ones-column appended to v so the same matmuls accumulate the softmax
    row-sums for free; final 1/rowsum scale on VectorE.
Matmuls use float32r (full PE rate for moving dim >= 256, ~1e-4 matmul
rel-err vs 2e-3 for bf16).
"""

import math

import numpy as np

B, S, D, H, O = 16, 2048, 512, 128, 512
N_CORES = 8
B_PER_CORE = B // N_CORES
P = 128          # partitions
CHUNK = 256      # sq chunk width for the attention phase
STAGE = 4        # s-tiles per x-load DMA group

_cache = {}


def _build_nc(with_bias: bool, b_per_core: int, s_tiles: int):
    import concourse.bass as bass
    import concourse.mybir as mybir
    import concourse.tile as tile
    from concourse import bacc
    from concourse.masks import make_identity

    f32 = mybir.dt.float32
    f32r = mybir.dt.float32r
    AF = mybir.ActivationFunctionType
    Alu = mybir.AluOpType
    Ax = mybir.AxisListType

    s_len = s_tiles * P
    d_tiles = D // P
    n_chunks = s_len // CHUNK
    subs = CHUNK // P
    assert s_tiles % STAGE == 0 and s_len % CHUNK == 0

    nc = bacc.Bacc("TRN2", target_bir_lowering=False, debug=False,
                   num_devices=N_CORES)
    x_d = nc.dram_tensor("x", [b_per_core, s_len, D], f32, kind="ExternalInput")
    wqt_d = nc.dram_tensor("wqt", [P, d_tiles, H], f32, kind="ExternalInput")
    wkt_d = nc.dram_tensor("wkt", [P, d_tiles, H], f32, kind="ExternalInput")
    wvt_d = nc.dram_tensor("wvt", [P, d_tiles, O], f32, kind="ExternalInput")
    if with_bias:
        bq_d = nc.dram_tensor("bq", [H, 1], f32, kind="ExternalInput")
        bk_d = nc.dram_tensor("bk", [H, 1], f32, kind="ExternalInput")
        bv_d = nc.dram_tensor("bv", [1, O], f32, kind="ExternalInput")
    out_d = nc.dram_tensor("out", [b_per_core, s_len, O], f32,
                           kind="ExternalOutput")

    with tile.TileContext(nc) as tc:
        from contextlib import ExitStack
        with ExitStack() as ctx:
            consts = ctx.enter_context(tc.tile_pool(name="consts", bufs=1))
            stagep = ctx.enter_context(tc.tile_pool(name="stage", bufs=3))
            ttp = ctx.enter_context(tc.tile_pool(name="tt", bufs=1))
            qkp = ctx.enter_context(tc.tile_pool(name="qk", bufs=1))
            vp = ctx.enter_context(tc.tile_pool(name="v", bufs=1))
            ptp = ctx.enter_context(tc.tile_pool(name="pt", bufs=3))
            outp = ctx.enter_context(tc.tile_pool(name="outsb", bufs=1))
            small = ctx.enter_context(tc.tile_pool(name="small", bufs=4))
            psA = ctx.enter_context(tc.tile_pool(name="psA", bufs=4, space="PSUM"))
            psO = ctx.enter_context(tc.tile_pool(name="psO", bufs=2, space="PSUM"))

            eps_t = consts.tile([P, 1], f32)
            nc.vector.memset(eps_t[:], 1e-5)
            ident = consts.tile([P, P], f32)
            make_identity(nc, ident[:])
            # f32r matmul operands must come from a rounding instruction, so
            # weights are DMA'd into staging and rounded into their tiles.
            wqt = consts.tile([P, d_tiles, H], f32)
            wkt = consts.tile([P, d_tiles, H], f32)
            wvt = consts.tile([P, d_tiles, O], f32)
            for w_t, w_d in ((wqt, wqt_d), (wkt, wkt_d), (wvt, wvt_d)):
                wst = stagep.tile([P, d_tiles, O], f32, tag="stage")
                fs = w_t.shape[2]
                nc.sync.dma_start(out=wst[:, :, 0:fs], in_=w_d[:])
                nc.vector.tensor_copy(out=w_t[:].bitcast(f32r),
                                      in_=wst[:, :, 0:fs])
            if with_bias:
                bq_t = consts.tile([H, 1], f32)
                nc.sync.dma_start(out=bq_t[:], in_=bq_d[:])
                bk_t = consts.tile([H, 1], f32)
                nc.sync.dma_start(out=bk_t[:], in_=bk_d[:])
                bv_t = consts.tile([P, O], f32)
                nc.sync.dma_start(out=bv_t[:], in_=bv_d[:].broadcast_to((P, O)))

            for b in range(b_per_core):
                xb = x_d[b].rearrange("(t p) d -> p t d", p=P)
                ob = out_d[b].rearrange("(t p) o -> p t o", p=P)

                tT = ttp.tile([P, d_tiles, s_len], f32, tag="tt")
                qT = qkp.tile([P, s_len], f32, tag="q")
                kT = qkp.tile([P, s_len], f32, tag="k")
                # v with an extra all-ones column: the attn@v matmul then
                # accumulates the softmax row-sums for free.
                # two ones-columns keep the second attn@v matmul's moving
                # dim even (odd free dims fail the f32r ISA check)
                v_sb = vp.tile([P, s_tiles, O + 2], f32, tag="v")
                vones = small.tile([P, s_tiles, 2], f32, tag="vones")
                nc.vector.memset(vones[:], 1.0)
                nc.vector.tensor_copy(out=v_sb[:, :, O:O + 2].bitcast(f32r),
                                      in_=vones[:])
                out_sb = outp.tile([P, s_tiles, O], f32, tag="o")

                # ---- Phase A: load, LayerNorm, transpose, projections ----
                for g in range(s_tiles // STAGE):
                    stg = stagep.tile([P, STAGE, D], f32, tag="stage")
                    nc.sync.dma_start(out=stg[:],
                                      in_=xb[:, g * STAGE:(g + 1) * STAGE, :])
                    for j in range(STAGE):
                        st = g * STAGE + j
                        sv = stg[:, j, :]
                        stats = small.tile([P, 6], f32, tag="stats")
                        nc.vector.bn_stats(out=stats[:], in_=sv)
                        mv = small.tile([P, 2], f32, tag="mv")
                        nc.vector.bn_aggr(out=mv[:], in_=stats[:])
                        rstd = small.tile([P, 1], f32, tag="rstd")
                        nc.scalar.activation(out=rstd[:], in_=mv[:, 1:2],
                                             func=AF.Sqrt, bias=eps_t[:])
                        nc.vector.reciprocal(out=rstd[:], in_=rstd[:])
                        nc.vector.tensor_scalar(
                            out=sv, in0=sv, scalar1=mv[:, 0:1], scalar2=rstd[:],
                            op0=Alu.subtract, op1=Alu.mult)
                        for dt in range(d_tiles):
                            ps = psA.tile([P, P], f32, tag="a")
                            nc.tensor.transpose(
                                ps[:], sv[:, dt * P:(dt + 1) * P], ident[:])
                            nc.vector.tensor_copy(
                                out=tT[:, dt, st * P:(st + 1) * P].bitcast(f32r),
                                in_=ps[:])

                # qT / kT projections (chunks of 512 along s)
                for c in range(s_len // 512):
                    cs = slice(c * 512, (c + 1) * 512)
                    for name, w_t, dst in (("q", wqt, qT), ("k", wkt, kT)):
                        ps = psA.tile([P, 512], f32, tag="a")
                        for dt in range(d_tiles):
                            nc.tensor.matmul(
                                ps[:], w_t[:, dt, :].bitcast(f32r),
                                tT[:, dt, cs].bitcast(f32r),
                                start=(dt == 0), stop=(dt == d_tiles - 1))
                        if with_bias:
                            bias = bq_t if name == "q" else bk_t
                            nc.vector.tensor_scalar_add(
                                out=dst[:, cs].bitcast(f32r), in0=ps[:],
                                scalar1=bias[:])
                        else:
                            nc.vector.tensor_copy(out=dst[:, cs].bitcast(f32r),
                                                  in_=ps[:])

                # v projection
                for st in range(s_tiles):
                    ps = psA.tile([P, O], f32, tag="a")
                    for dt in range(d_tiles):
                        nc.tensor.matmul(
                            ps[:], tT[:, dt, st * P:(st + 1) * P].bitcast(f32r),
                            wvt[:, dt, :].bitcast(f32r),
                            start=(dt == 0), stop=(dt == d_tiles - 1))
                    nc.vector.tensor_copy(out=v_sb[:, st, 0:O].bitcast(f32r), in_=ps[:])

                # ---- Phase B: attention over sq chunks ----
                HALF = O // 2
                for c in range(n_chunks):
                    cq = slice(c * CHUNK, (c + 1) * CHUNK)
                    pT = ptp.tile([P, s_tiles, CHUNK], f32, tag="pt")
                    for i in range(s_tiles):
                        sps = psA.tile([P, CHUNK], f32, tag="a")
                        nc.tensor.matmul(
                            sps[:], kT[:, i * P:(i + 1) * P].bitcast(f32r),
                            qT[:, cq].bitcast(f32r), start=True, stop=True)
                        nc.scalar.activation(out=pT[:, i, :].bitcast(f32r),
                                             in_=sps[:], func=AF.Exp)
                    for sub in range(subs):
                        st = c * subs + sub
                        # o split 256 + 257: the 257th column of the second
                        # half is p @ ones = the softmax row-sum.
                        ops_a = psO.tile([P, HALF], f32, tag="oa")
                        ops_b = psO.tile([P, HALF + 2], f32, tag="ob")
                        for i in range(s_tiles):
                            lhsT = pT[:, i, sub * P:(sub + 1) * P].bitcast(f32r)
                            nc.tensor.matmul(
                                ops_a[:], lhsT,
                                v_sb[:, i, 0:HALF].bitcast(f32r),
                                start=(i == 0), stop=(i == s_tiles - 1))
                            nc.tensor.matmul(
                                ops_b[:], lhsT,
                                v_sb[:, i, HALF:O + 2].bitcast(f32r),
                                start=(i == 0), stop=(i == s_tiles - 1))
                        recip = small.tile([P, 1], f32, tag="recip")
                        nc.vector.reciprocal(out=recip[:],
                                             in_=ops_b[:, HALF:HALF + 1])
                        nc.vector.tensor_scalar_mul(
                            out=out_sb[:, st, 0:HALF], in0=ops_a[:],
                            scalar1=recip[:])
                        nc.vector.tensor_scalar_mul(
                            out=out_sb[:, st, HALF:O], in0=ops_b[:, 0:HALF],
                            scalar1=recip[:])
                        if with_bias:
                            nc.vector.tensor_add(out=out_sb[:, st, :],
                                                 in0=out_sb[:, st, :],
                                                 in1=bv_t[:])
                    # store per half-batch
                    if (c + 1) % (n_chunks // 2) == 0:
                        h0 = (s_tiles // 2) * ((c + 1) // (n_chunks // 2) - 1)
                        hs = slice(h0, h0 + s_tiles // 2)
                        nc.sync.dma_start(out=ob[:, hs, :],
                                          in_=out_sb[:, hs, :])
    nc.compile()
    return nc


def _get_runner(with_bias: bool, b_per_core: int = B_PER_CORE,
                s_tiles: int = S // P):
    key = (with_bias, b_per_core, s_tiles)
    if key in _cache:
        return _cache[key]

    import jax
    import jax.numpy as jnp  # noqa: F401
    import concourse.mybir as mybir
    from concourse.bass2jax import (
        _bass_exec_p, install_neuronx_cc_hook, partition_id_tensor)
    from jax.experimental.shard_map import shard_map
    from jax.sharding import Mesh, PartitionSpec

    nc = _build_nc(with_bias, b_per_core, s_tiles)
    install_neuronx_cc_hook()
    partition_name = (nc.partition_id_tensor.name
                      if nc.partition_id_tensor else None)

    in_names, out_names, out_avals = [], [], []
    for alloc in nc.m.functions[0].allocations:
        if not isinstance(alloc, mybir.MemoryLocationSet):
            continue
        name = alloc.memorylocations[0].name
        if alloc.kind == "ExternalInput":
            if name != partition_name:
                in_names.append(name)
        elif alloc.kind == "ExternalOutput":
            out_names.append(name)
            out_avals.append(jax.core.ShapedArray(
                tuple(alloc.tensor_shape), mybir.dt.np(alloc.dtype)))
    n_params = len(in_names)
    all_in_names = list(in_names) + list(out_names)
    if partition_name is not None:
        all_in_names.append(partition_name)
    donate = tuple(range(n_params, n_params + len(out_names)))

    def _body(*args):
        operands = list(args)
        if partition_name is not None:
            operands.append(partition_id_tensor())
        outs = _bass_exec_p.bind(
            *operands, out_avals=tuple(out_avals),
            in_names=tuple(all_in_names),
            out_names=tuple(out_names), lowering_input_output_aliases=(),
            sim_require_finite=True, sim_require_nnan=True, nc=nc)
        return tuple(outs)

    devices = jax.devices()[:N_CORES]
    mesh = Mesh(np.asarray(devices), ("core",))
    nin = n_params + len(out_names)
    sharded = jax.jit(
        shard_map(_body, mesh=mesh, in_specs=(PartitionSpec("core"),) * nin,
                  out_specs=(PartitionSpec("core"),) * len(out_names),
                  check_rep=False),
        donate_argnums=donate, keep_unused=True)

    runner = (sharded, in_names, out_names, out_avals)
    _cache[key] = runner
    return runner


def _prep_weights(ln_w, ln_b, Wq, Wk, Wv):
    scale = np.float32(1.0 / math.sqrt(O))
    wq_eff = (Wq * ln_w[None, :]).astype(np.float32) * scale
    wk_eff = (Wk * ln_w[None, :]).astype(np.float32)
    wv_eff = (Wv * ln_w[None, :]).astype(np.float32)
    bq = (Wq.astype(np.float64) @ ln_b.astype(np.float64)).astype(np.float32) * scale
    bk = (Wk.astype(np.float64) @ ln_b.astype(np.float64)).astype(np.float32)
    bv = (Wv.astype(np.float64) @ ln_b.astype(np.float64)).astype(np.float32)
    with_bias = bool(np.any(bq) or np.any(bk) or np.any(bv))

    d_tiles = D // P
    def pack(w_eff, n_out):  # [n_out, D] -> [P, d_tiles, n_out]
        return np.ascontiguousarray(
            w_eff.T.reshape(d_tiles, P, n_out).transpose(1, 0, 2))

    packed = {
        "wqt": pack(wq_eff, H),
        "wkt": pack(wk_eff, H),
        "wvt": pack(wv_eff, O),
    }
    if with_bias:
        packed["bq"] = bq.reshape(H, 1)
        packed["bk"] = bk.reshape(H, 1)
        packed["bv"] = bv.reshape(1, O)
    return packed, with_bias


def run_device(x_full, packed, with_bias, b_per_core=B_PER_CORE,
               s_tiles=S // P):
    """x_full: [n_cores*b_per_core, s, D] -> out of same shape."""
    sharded, in_names, out_names, out_avals = _get_runner(
        with_bias, b_per_core, s_tiles)
    per_input = {"x": np.ascontiguousarray(x_full, dtype=np.float32)}
    for name, arr in packed.items():
        per_input[name] = np.concatenate([arr] * N_CORES, axis=0)
    args = [per_input[n] for n in in_names]
    zero_outs = [np.zeros((N_CORES * a.shape[0], *a.shape[1:]), a.dtype)
                 for a in out_avals]
    out_arrs = sharded(*args, *zero_outs)
    out = np.asarray(out_arrs[out_names.index("out")])
    return out.reshape(x_full.shape[0], x_full.shape[1], O)


def kernel(x, ln_w, ln_b, Wq, Wk, Wv):
    x = np.asarray(x, dtype=np.float32)
    packed, with_bias = _prep_weights(
        np.asarray(ln_w, np.float32), np.asarray(ln_b, np.float32),
        np.asarray(Wq, np.float32), np.asarray(Wk, np.float32),
        np.asarray(Wv, np.float32))
    return run_device(x, packed, with_bias)
